# revision 24
# baseline (speedup 1.0000x reference)
"""Trainium2 Bass kernel for nn_Attention_26336739459136.

Reference computation (all fp32):
    t = s + TG_prompt                        # [4096, 1024]
    c = concat([t, query])                   # [8192, 1024]
    q, k, v = split(c @ W_qkv.T)             # v is UNUSED
    attn = softmax((q*S) @ (k.T*S))          # S = 1024**-0.25, full 8192x8192
    x_s = (attn[:4096, 4096:] @ query) @ W_proj_s.T
    x_q = (attn[4096:, :4096] @ s) @ W_proj_q.T
    return (x_s, x_q)

Sharding: every core owns an interleaved slice of 512 s-rows + 512 q-rows
(rows [512m, 512(m+1)) of each block), so the SPMD program is identical on
all 8 cores.  Per core:
  phase 0a: qT/kT projection of its 1024 rows (contraction layouts come in
            pre-transposed from the host).  q gets the combined 1/32 scale.
  AG#1:     AllGather of kT shard -> full kT [8192 rows of d, 1024].
  phase 0b: value' shards: q'_m = query_m @ W_proj_s.T, s'_m = s_m @ W_proj_q.T
            (projection folded through the attention matmul by associativity).
  AG#2:     AllGather of [q'_m; s'_m] -> v_all (overlaps phase 1).
  phase 1:  scores for its 1024 rows vs all 8192 keys, no max subtraction
            (scores are ~N(0, 2), exp can't overflow fp32).  Numerator-side
            blocks are computed transposed ([j, i]) and exp'd to bf16 tiles;
            denominator-only blocks are computed [i, j] and folded to
            row-sum partials via the activation accumulator.
  phase 2:  numerator = E @ v' accumulated over 32 key tiles in PSUM, plus a
            ones-column matmul for the numerator-side part of the row sums.
            Normalize by 1/rowsum (per-partition scalar) on evacuation.
"""

import numpy as np

import concourse.bass as bass
import concourse.bacc as bacc
import concourse.mybir as mybir
import concourse.tile as tile
from concourse.bass_utils import run_bass_kernel_spmd

P = 128
C = 1024          # model dim (contraction for projections)
D = 1024          # head dim (contraction for scores)
NCORES = 8
HALF = 512        # rows of each branch owned per core
ROWS = 1024       # total rows owned per core
SCALE2 = float(C) ** -0.5   # (C**-0.25)**2 applied once to q

F32 = mybir.dt.float32
BF16 = mybir.dt.bfloat16
AX = mybir.AxisListType.X
EXP = mybir.ActivationFunctionType.Exp
COPY = mybir.ActivationFunctionType.Copy

# dtype of the score matmuls (q/k stay fp32 through the projection)
F32R = mybir.dt.float32r
SCORE_DT = F32R


def _r(ap):
    """[N*128, F] dram view -> [128, N, F] partition-major tiles."""
    return ap.rearrange("(o p) f -> p o f", p=P)


def build_program():
    nc = bacc.Bacc(
        "TRN2", target_bir_lowering=False, debug=False, num_devices=NCORES
    )

    # ---- I/O ----
    aT = nc.dram_tensor("aT", [C, ROWS], F32R, kind="ExternalInput")
    tgT = nc.dram_tensor("tgT", [C, HALF], F32, kind="ExternalInput")
    wqkT = nc.dram_tensor("wqkT", [C, 2 * D], F32R, kind="ExternalInput")
    wpsT = nc.dram_tensor("wpsT", [D, D], F32R, kind="ExternalInput")
    wpqT = nc.dram_tensor("wpqT", [D, D], F32R, kind="ExternalInput")
    out_s = nc.dram_tensor("out_s", [HALF, D], F32, kind="ExternalOutput")
    out_q = nc.dram_tensor("out_q", [HALF, D], F32, kind="ExternalOutput")

    # ---- collective buffers ----
    kt_in = nc.dram_tensor("kt_in", [D, ROWS], F32R, kind="Internal")
    kt_all = nc.dram_tensor(
        "kt_all", [NCORES * D, ROWS], F32R, kind="Internal", addr_space="Shared"
    )
    v_in = nc.dram_tensor("v_in", [ROWS, D], BF16, kind="Internal")
    v_all = nc.dram_tensor(
        "v_all", [NCORES * ROWS, D], BF16, kind="Internal", addr_space="Shared"
    )

    with tile.TileContext(nc) as tc:
        with tc.tile_pool(name="persist", bufs=1) as persist:
            # qT [d, i] for own 1024 rows, fp32, lives through phase 1
            qT = persist.tile([P, D // P, ROWS], SCORE_DT)
            # denominator partials: [p, i-tile(8), rank(8)]
            dparts = persist.tile([P, 8 * NCORES], F32)
            ones_sb = persist.tile([P, 1], BF16)
            nc.vector.memset(ones_sb[:], 1.0)

            # ================= phase 0a: q/k projection =================
            with (
                tc.tile_pool(name="ph0", bufs=1) as ph0,
                tc.tile_pool(name="ph0w", bufs=3) as ph0w,
                tc.tile_pool(name="ph0s", bufs=3) as ph0s,
                tc.tile_pool(name="psum0", bufs=6, space="PSUM") as psum0,
            ):
                aT_sb = ph0.tile([P, C // P, ROWS], F32R)
                nc.sync.dma_start(aT_sb[:], _r(aT[:]))

                # cT for the s-row half = aT[:, :512] + tgT (q-half is raw aT)
                cT_half = ph0.tile([P, C // P, HALF], F32R)
                with tc.tile_pool(name="tg", bufs=1) as tgp:
                    tgT_sb = tgp.tile([P, C // P, HALF], F32)
                    nc.scalar.dma_start(tgT_sb[:], _r(tgT[:]))
                    for ct in range(C // P):
                        nc.vector.tensor_add(
                            cT_half[:, ct], aT_sb[:, ct, 0:HALF], tgT_sb[:, ct]
                        )

                def rhs_c(ct, ic):
                    if ic == 0:
                        return cT_half[:, ct]
                    return aT_sb[:, ct, HALF:ROWS]

                # ========== q/k projection: k first so AG#1 launches early ==========
                for dt_i in list(range(D // P, 2 * D // P)) + list(range(D // P)):
                    wt = ph0w.tile([P, C // P, P], F32R, tag="wqk", bufs=3)
                    nc.scalar.dma_start(
                        wt[:], _r(wqkT[:, dt_i * P : (dt_i + 1) * P])
                    )
                    # q-half (raw aT) first: PE starts before the TG add chain
                    for ic in (1, 0):
                        ps = psum0.tile([P, HALF], F32, tag="ps0")
                        for ct in range(C // P):
                            nc.tensor.matmul(
                                ps[:],
                                (wt[:, ct]),
                                (rhs_c(ct, ic)),
                                start=(ct == 0),
                                stop=(ct == C // P - 1),
                            )
                        if dt_i < D // P:
                            nc.scalar.activation(
                                qT[:, dt_i, ic * HALF : (ic + 1) * HALF],
                                ps[:],
                                COPY,
                                scale=SCALE2,
                            )
                        else:
                            kout = ph0s.tile([P, HALF], SCORE_DT, tag="kout")
                            nc.scalar.copy(kout[:], ps[:])
                            kd = dt_i - D // P
                            nc.sync.dma_start(
                                kt_in[
                                    kd * P : (kd + 1) * P,
                                    ic * HALF : (ic + 1) * HALF,
                                ],
                                kout[:],
                            )
                    if dt_i == 2 * D // P - 1:
                        # all k tiles written -> AG#1 overlaps everything after
                        nc.gpsimd.collective_compute(
                            "AllGather",
                            mybir.AluOpType.bypass,
                            replica_groups=[list(range(NCORES))],
                            ins=[kt_in[:].opt()],
                            outs=[kt_all[:].opt()],
                        )

                # ============= value' shards (RAW s/query slices) =============
                # q'_m = query_m @ wps.T : lhsT = queryT slice (aT cols 512:)
                # s'_m = s_m @ wpq.T     : lhsT = sT slice (aT cols 0:512)
                vall_sb = ph0.tile([P, 8, D], BF16)
                for half_i, (lo, w_dram) in enumerate([(HALF, wpsT), (0, wpqT)]):
                    for ec in range(2):
                        wt = ph0w.tile([P, C // P, HALF], F32R, tag="wp", bufs=2)
                        nc.scalar.dma_start(
                            wt[:], _r(w_dram[:, ec * HALF : (ec + 1) * HALF])
                        )
                        for jt in range(HALF // P):
                            ps = psum0.tile([P, HALF], F32, tag="ps0")
                            for ct in range(C // P):
                                nc.tensor.matmul(
                                    ps[:],
                                    (aT_sb[:, ct, lo + jt * P : lo + (jt + 1) * P]),
                                    (wt[:, ct]),
                                    start=(ct == 0),
                                    stop=(ct == C // P - 1),
                                )
                            nc.scalar.copy(
                                vall_sb[
                                    :,
                                    half_i * 4 + jt,
                                    ec * HALF : (ec + 1) * HALF,
                                ],
                                ps[:],
                            )
                nc.sync.dma_start(_r(v_in[:]), vall_sb[:])

                # AG#2: value' (needed only by phase 2; overlaps phase 1)
                nc.gpsimd.collective_compute(
                    "AllGather",
                    mybir.AluOpType.bypass,
                    replica_groups=[list(range(NCORES))],
                    ins=[v_in[:].opt()],
                    outs=[v_all[:].opt()],
                )

            # ================= phases 1+2 =================
            with tc.tile_pool(name="epool", bufs=1) as epool:
                # exp'd transposed numerator scores, bf16:
                #  eC: j = q-rows (8 ranks x 4 jt), i = own s-rows
                #  eD: j = s-rows,                  i = own q-rows
                eC = epool.tile([P, 32, HALF], BF16)
                eD = epool.tile([P, 32, HALF], BF16)
                phase_12(nc, tc, qT, eC, eD, dparts, ones_sb, kt_all, v_all, out_s, out_q)
    nc.compile()
    return nc


def phase_12(nc, tc, qT, eC, eD, dparts, ones_sb, kt_all, v_all, out_s, out_q):
            # ================= phase 1: scores + exp =================
            with (
                tc.tile_pool(name="kt", bufs=2) as ktp,
                tc.tile_pool(name="sc", bufs=4) as scp,
                tc.tile_pool(name="psum1", bufs=6, space="PSUM") as psum1,
            ):
                for r in range(NCORES):
                    ktile = ktp.tile([P, D // P, ROWS], SCORE_DT, tag="kt")
                    nc.sync.dma_start(
                        ktile[:], _r(kt_all[r * D : (r + 1) * D, :])
                    )
                    # paths A/B: denominator-only blocks, [i, j] layout
                    for it in range(8):
                        jlo = 0 if it < 4 else HALF  # s-rows vs own-kind keys
                        ps = psum1.tile([P, HALF], F32, tag="ps1")
                        for dd in range(D // P):
                            nc.tensor.matmul(
                                ps[:],
                                (qT[:, dd, it * P : (it + 1) * P]),
                                (ktile[:, dd, jlo : jlo + HALF]),
                                start=(dd == 0),
                                stop=(dd == D // P - 1),
                            )
                        junk = scp.tile([P, HALF], BF16, tag="junk")
                        nc.scalar.activation(
                            junk[:], ps[:], EXP, accum_out=dparts[:, it * NCORES + r : it * NCORES + r + 1]
                        )
                    # paths C/D: numerator blocks, [j, i] layout -> bf16 E
                    for path_i, (jlo, ilo, e_sb) in enumerate(
                        [(HALF, 0, eC), (0, HALF, eD)]
                    ):
                        for jt in range(4):
                            ps = psum1.tile([P, HALF], F32, tag="ps1")
                            for dd in range(D // P):
                                nc.tensor.matmul(
                                    ps[:],
                                    (ktile[:, dd, jlo + jt * P : jlo + (jt + 1) * P]),
                                    (qT[:, dd, ilo : ilo + HALF]),
                                    start=(dd == 0),
                                    stop=(dd == D // P - 1),
                                )
                            nc.scalar.activation(
                                e_sb[:, r * 4 + jt], ps[:], EXP
                            )

            # ================= phase 2: numerator + normalize =================
            with (
                tc.tile_pool(name="vv", bufs=1) as vvp,
                tc.tile_pool(name="fin", bufs=3) as finp,
                tc.tile_pool(name="psum2", bufs=2, space="PSUM") as psum2,
                tc.tile_pool(name="psum2o", bufs=2, space="PSUM") as psum2o,
            ):
                # v_all rows decompose as (rank r, half h, jt, p); pick half h
                for half_i, (e_sb, out_t) in enumerate([(eC, out_s), (eD, out_q)]):
                    # value tiles for this half: q' blocks for s-rows, s' for q-rows
                    vsb = vvp.tile([P, 32, D], BF16, tag="v")
                    for r in range(NCORES):
                        nc.sync.dma_start(
                            vsb[:, r * 4 : (r + 1) * 4, :],
                            _r(
                                v_all[
                                    r * ROWS + half_i * HALF : r * ROWS
                                    + half_i * HALF
                                    + HALF,
                                    :,
                                ]
                            ),
                        )
                    for it in range(4):
                        it_g = half_i * 4 + it  # global i-tile for dparts
                        psA = psum2.tile([P, HALF], F32, tag="psA")
                        psB = psum2.tile([P, HALF], F32, tag="psB")
                        psO = psum2o.tile([P, 1], F32, tag="psO")
                        for j in range(32):
                            lhsT = e_sb[:, j, it * P : (it + 1) * P]
                            st = dict(start=(j == 0), stop=(j == 31))
                            nc.tensor.matmul(psA[:], lhsT, vsb[:, j, 0:HALF], **st)
                            nc.tensor.matmul(psB[:], lhsT, vsb[:, j, HALF:D], **st)
                            nc.tensor.matmul(psO[:], lhsT, ones_sb[:], **st)
                        dsum = finp.tile([P, 1], F32, tag="dsum")
                        nc.vector.reduce_sum(dsum[:], dparts[:, it_g * NCORES : (it_g + 1) * NCORES], axis=AX)
                        nc.vector.tensor_add(dsum[:], dsum[:], psO[:])
                        recip = finp.tile([P, 1], F32, tag="recip")
                        nc.vector.reciprocal(recip[:], dsum[:])
                        otile = finp.tile([P, D], F32, tag="otile")
                        nc.scalar.activation(
                            otile[:, 0:HALF], psA[:], COPY, scale=recip[:]
                        )
                        nc.scalar.activation(
                            otile[:, HALF:D], psB[:], COPY, scale=recip[:]
                        )
                        nc.sync.dma_start(
                            out_t[it * P : (it + 1) * P, :], otile[:]
                        )


_NC_CACHE = None


def kernel(query, s, TG_prompt, W_qkv, W_proj_s, W_proj_q):
    global _NC_CACHE
    query = np.asarray(query, dtype=np.float32)
    s = np.asarray(s, dtype=np.float32)
    TG_prompt = np.asarray(TG_prompt, dtype=np.float32)
    W_qkv = np.asarray(W_qkv, dtype=np.float32)
    W_proj_s = np.asarray(W_proj_s, dtype=np.float32)
    W_proj_q = np.asarray(W_proj_q, dtype=np.float32)

    sT = np.ascontiguousarray(s.T)
    qryT = np.ascontiguousarray(query.T)
    tgT_full = np.ascontiguousarray(TG_prompt.T)
    wqkT = np.ascontiguousarray(W_qkv[: 2 * D].T)
    wpsT = np.ascontiguousarray(W_proj_s.T)
    wpqT = np.ascontiguousarray(W_proj_q.T)

    if _NC_CACHE is None:
        _NC_CACHE = build_program()
    nc = _NC_CACHE

    in_maps = []
    for m in range(NCORES):
        sl = slice(m * HALF, (m + 1) * HALF)
        in_maps.append(
            {
                "aT": np.ascontiguousarray(
                    np.concatenate([sT[:, sl], qryT[:, sl]], axis=1)
                ),
                "tgT": np.ascontiguousarray(tgT_full[:, sl]),
                "wqkT": wqkT,
                "wpsT": wpsT,
                "wpqT": wpqT,
            }
        )

    res = run_bass_kernel_spmd(nc, in_maps, core_ids=list(range(NCORES)))
    outs = res.results

    x_s = np.concatenate([outs[m]["out_s"] for m in range(NCORES)], axis=0)
    x_q = np.concatenate([outs[m]["out_q"] for m in range(NCORES)], axis=0)
    return (x_s, x_q)
